# revision 35
# baseline (speedup 1.0000x reference)
"""GGNN (gated graph NN) message-passing kernel for 8 Trainium2 NeuronCores.

Sharding: edge-type sharding. Core c owns edge-type block c of the adjacency
matrix (columns c*N..(c+1)*N of the [N, 2E*N] adjacency, pre-transposed on the
host) plus node shard c for the GRU update.

Per step, on core c (node shard split in halves A = block {128c},
B = block {1024+128c} — the blocks the two ReduceScatters deliver):
  prestage: z/r U-term matmuls (local h only) fill the AG-A window
  stage1: t_c = h @ W_prop[c]                       [N, D]
          (half-A m-tiles emitted first so they only wait on AG-A;
           half-B m-tiles wait on AG-B)
  stage2: partial_a_c = A_cT.T @ t_c                [N, D]  in 4 sub-groups
          of 4 node-tiles (4 PSUM banks), each split into contraction
          phase1 (m=0..7, needs only stage1-A — overlaps in-flight AG-B)
          and phase2 (m=8..15, adds SBUF-staged phase-1 partials on the
          way out). Emission order: s1A, ph1(0,1), s1B, ph2(0) ,ph2(1)
          [issue RS-A], ph1(2,3), ph2(2), ph2(3) [issue RS-B].
  tail, per half X in {A, B}:
      transpose a_X -> aT_X [D, 128]
      GRU gates on half X (fp16 matmuls; z/r U-terms come from the
      prestage via a PSUM-preload identity matmul)
      h'_X elementwise
      AG-X: AllGather(h'_X^T) -> [8*D, 128]
  Half-A gates run while RS-B is in flight; AG-A runs while half-B gates
  compute; AG-B overlaps next step's prestage + stage1-A + stage2-phase1.

Numerics: all matmuls in float16 (full PE rate at any free size; adjacency
0/1 is exact in fp16, weights/states lose ~2^-11 relative); collective
payloads fp16; accumulation fp32 in PSUM; elementwise GRU update in fp32.
"""
import sys
if "/opt/trn_rl_repo" not in sys.path:
    sys.path.insert(0, "/opt/trn_rl_repo")

import numpy as np
import ml_dtypes

NC_CORES = 8
N = 2048          # nodes
D = 512           # state dim
ANN = 256         # annotation dim
STEPS = 5
SH = N // NC_CORES   # 256 nodes per shard
HH = SH // 2         # 128 nodes per half-shard
KT = D // 128        # 4
MT = N // 128        # 16


def build(repeats=1, ablate=()):
    import concourse.bacc as bacc
    import concourse.mybir as mybir
    import concourse.tile as tile
    from concourse.masks import make_identity

    dt = mybir.dt
    nc = bacc.Bacc()
    at_p = nc.declare_dram_parameter("at", [N, N], dt.float16, isOutput=False)
    h0t_p = nc.declare_dram_parameter("h0t", [NC_CORES * D, SH], dt.float16,
                                      isOutput=False)
    h0sr_p = nc.declare_dram_parameter("h0sr", [D, SH], dt.float16, isOutput=False)
    h0s_p = nc.declare_dram_parameter("h0s", [D, SH], dt.float32, isOutput=False)
    wc_p = nc.declare_dram_parameter("wc", [D, D], dt.float16, isOutput=False)
    gw_p = nc.declare_dram_parameter("gw", [6, D, D], dt.float16, isOutput=False)
    bpc_p = nc.declare_dram_parameter("bpc", [1, D], dt.float32, isOutput=False)
    bz_p = nc.declare_dram_parameter("bzc", [D, 1], dt.float32, isOutput=False)
    br_p = nc.declare_dram_parameter("brc", [D, 1], dt.float32, isOutput=False)
    bh_p = nc.declare_dram_parameter("bhc", [D, 1], dt.float32, isOutput=False)
    out_p = nc.declare_dram_parameter("out", [D, SH], dt.float32, isOutput=True)
    RG = [list(range(NC_CORES))]

    from contextlib import ExitStack
    with tile.TileContext(nc) as tc, ExitStack() as stk:
        res = stk.enter_context(tc.tile_pool(name="res", bufs=1))
        # PSUM: bank-granular (8 banks). stage1/stage2 chains use p_mm (5),
        # gate chains + transposes use p_g (3).
        p_mm = stk.enter_context(tc.tile_pool(name="pmm", bufs=5, space="PSUM"))
        p_g = stk.enter_context(tc.tile_pool(name="pg", bufs=3, space="PSUM"))
        p_hc = stk.enter_context(tc.tile_pool(name="phc", bufs=6))
        p_t = stk.enter_context(tc.tile_pool(name="pt", bufs=1))
        p_pp = stk.enter_context(tc.tile_pool(name="ppp", bufs=1))
        p_asb = stk.enter_context(tc.tile_pool(name="pasb", bufs=4))
        p_sm = stk.enter_context(tc.tile_pool(name="psm", bufs=2))
        p_h = stk.enter_context(tc.tile_pool(name="ph", bufs=2))
        dram = stk.enter_context(tc.tile_pool(name="dram", bufs=2, space="DRAM"))

        # ---- setup: constants, weights, adjacency ----
        identity = res.tile([128, 128], dt.float32, tag="identity")
        make_identity(nc, identity[:])
        identity16 = res.tile([128, 128], dt.float16, tag="identity16")
        nc.vector.tensor_copy(identity16[:], identity[:])
        ones = res.tile([1, 128], dt.float32, tag="ones")
        nc.vector.memset(ones[:], 1.0)
        bpc_t = res.tile([1, D], dt.float32, tag="bpc")
        nc.sync.dma_start(bpc_t[:], bpc_p[:])
        pb = p_mm.tile([128, D], dt.float32, tag="mm")
        nc.tensor.matmul(pb[:], ones[:], bpc_t[:], start=True, stop=True)
        bias_bcast = res.tile([128, D], dt.float32, tag="bias_bcast")
        nc.vector.tensor_copy(bias_bcast[:], pb[:])

        bias_tiles = {}
        for nm, par in (("z", bz_p), ("r", br_p), ("h", bh_p)):
            for f in range(KT):
                bt = res.tile([128, 1], dt.float32, tag=f"b{nm}{f}")
                nc.sync.dma_start(bt[:], par[f * 128:(f + 1) * 128, :])
                bias_tiles[(nm, f)] = bt

        wc_t = []
        for k in range(KT):
            w = res.tile([128, D], dt.float16, tag=f"wc{k}")
            nc.sync.dma_start(w[:], wc_p[k * 128:(k + 1) * 128, :])
            wc_t.append(w)

        at_t = []
        for m in range(MT):
            a = res.tile([128, N], dt.float16, tag=f"at{m}")
            nc.sync.dma_start(a[:], at_p[m * 128:(m + 1) * 128, :])
            at_t.append(a)

        # resident GRU weights (fp16), loaded once
        gw_res = []
        for g in range(6):
            w = res.tile([128, KT, D], dt.float16, tag=f"gwr{g}")
            nc.scalar.dma_start(w[:], gw_p[g].rearrange("(k p) f -> p k f", p=128))
            gw_res.append(w)

        import concourse.mybir as _mb
        SIG = _mb.ActivationFunctionType.Sigmoid
        TANH = _mb.ActivationFunctionType.Tanh

        for rep in range(repeats):
          # step-0 h state
          hsh_prev = []   # h^T shard, fp16 (GRU U-term rhs)
          h32_prev = []   # h^T shard, fp32 (elementwise state)
          for k in range(KT):
            hr = p_h.tile([128, SH], dt.float16, tag=f"hnr{k}")
            nc.sync.dma_start(hr[:], h0sr_p[k * 128:(k + 1) * 128, :])
            hsh_prev.append(hr)
            h3 = p_h.tile([128, SH], dt.float32, tag=f"h32{k}")
            nc.sync.dma_start(h3[:], h0s_p[k * 128:(k + 1) * 128, :])
            h32_prev.append(h3)

          ag_prev = None   # pair (agA, agB) of [NC*D, HH] fp16

          for s in range(STEPS):
             # ---- pre-stage z/r U-terms (depend only on local h'^T, so
             # they fill the PE idle window while AG-A/AG-B are in flight;
             # consumed via a PSUM-preload identity matmul in gate_mm) ----
             uP = {}
             if "gru" not in ablate:
                 for gi, uidx in (("z", 1), ("r", 3)):
                     Uq = gw_res[uidx]
                     for X in range(2):
                         for f in range(KT):
                             pg = p_g.tile([128, HH], dt.float32, tag="gg",
                                           name=f"pu{gi}{X}{f}")
                             for k in range(KT):
                                 nc.tensor.matmul(
                                     pg[:], Uq[:, k, f * 128:(f + 1) * 128],
                                     hsh_prev[k][:, X * HH:(X + 1) * HH],
                                     start=(k == 0), stop=(k == KT - 1))
                             up = p_sm.tile([128, HH], dt.float16,
                                            tag=f"uP{gi}{X}{f}",
                                            name=f"uP{gi}{X}{f}")
                             if f % 2 == 0:
                                 nc.scalar.copy(up[:], pg[:])
                             else:
                                 nc.vector.tensor_copy(up[:], pg[:])
                             uP[(gi, X, f)] = up
             # ---- stage 1 + stage 2, software-pipelined ----
             # stage1 half-X m-tiles (mloc=X) only need AG-X of the previous
             # step. stage2 is split into two contraction phases: phase1
             # (m=0..7, needs only stage1-A) runs between stage1-A and
             # stage1-B so it overlaps the in-flight AG-B; phase2 (m=8..15)
             # adds the SBUF-staged phase-1 partials on the way out. This
             # moves RS-A's inputs ~15us earlier.
             t_tiles = [None] * MT

             def stage1(mloc):
                 for mp in range(8):
                     m = mp + 8 * mloc
                     if "s1" in ablate:
                         pt = p_mm.tile([128, D], dt.float32, tag="mm",
                                        name="pt")
                         nc.tensor.matmul(pt[:], wc_t[0][:, 0:128], wc_t[1][:],
                                          start=True, stop=True)
                     else:
                         hc = p_hc.tile([128, KT, 128], dt.float16, tag="hc",
                                        name="hc")
                         if s == 0:
                             blk = h0t_p[512 * mp:512 * (mp + 1),
                                         mloc * HH:(mloc + 1) * HH]
                         else:
                             blk = ag_prev[mloc][512 * mp:512 * (mp + 1), :]
                         nc.sync.dma_start(
                             hc[:], blk.rearrange("(k p) j -> p k j", p=128))
                         pt = p_mm.tile([128, D], dt.float32, tag="mm",
                                        name="pt")
                         for k in range(KT):
                             nc.tensor.matmul(pt[:], hc[:, k, :], wc_t[k][:],
                                              start=(k == 0), stop=(k == KT - 1))
                     tm = p_t.tile([128, D], dt.float16, tag=f"t{m}", name="tm")
                     nc.vector.tensor_add(tm[:], pt[:], bias_bcast[:])
                     t_tiles[m] = tm

             rs_ins = [dram.tile([N // 2, D], dt.float16, tag=f"rs_in{g}",
                                 name=f"rs_in{g}") for g in range(2)]
             rs_outs = []
             pp = {}

             def s2_phase1(sub):
                 if "s2" in ablate:
                     return
                 pas = [p_mm.tile([128, D], dt.float32, tag="mm",
                                  name=f"pa1_{sub}_{i}") for i in range(4)]
                 for m in range(8):
                     for i in range(4):
                         col = sub * 512 + i * 128
                         nc.tensor.matmul(pas[i][:],
                                          at_t[m][:, col:col + 128],
                                          t_tiles[m][:],
                                          start=(m == 0), stop=(m == 7))
                 for i in range(4):
                     ppt = p_pp.tile([128, D], dt.float32, tag=f"pp{sub}{i}",
                                     name=f"pp{sub}{i}")
                     if i % 2 == 0:
                         nc.scalar.copy(ppt[:], pas[i][:])
                     else:
                         nc.vector.tensor_copy(ppt[:], pas[i][:])
                     pp[(sub, i)] = ppt

             def s2_phase2(sub):
                 pas = [p_mm.tile([128, D], dt.float32, tag="mm",
                                  name=f"pa2_{sub}_{i}") for i in range(4)]
                 if "s2" in ablate:
                     for i in range(4):
                         nc.tensor.matmul(pas[i][:], t_tiles[0][:, 0:128],
                                          t_tiles[1][:], start=True, stop=True)
                 else:
                  for m in range(8, MT):
                     for i in range(4):
                         col = sub * 512 + i * 128
                         nc.tensor.matmul(pas[i][:],
                                          at_t[m][:, col:col + 128],
                                          t_tiles[m][:],
                                          start=(m == 8), stop=(m == MT - 1))
                 for i in range(4):
                     asb = p_asb.tile([128, D], dt.float16, tag="asb",
                                      name="asb")
                     if "s2" in ablate:
                         nc.vector.tensor_copy(asb[:], pas[i][:])
                     else:
                         nc.vector.tensor_add(asb[:], pas[i][:],
                                              pp[(sub, i)][:])
                     eng = nc.sync if i % 2 == 0 else nc.scalar
                     row = 512 * (sub % 2) + i * 128
                     eng.dma_start(rs_ins[sub // 2][row:row + 128, :], asb[:])
                 if sub % 2 == 1:
                     grp = sub // 2
                     # RS of this half: core c receives node block grp*1024+128c
                     rs_out = dram.tile([HH, D], dt.float16, tag=f"rs_out{grp}",
                                        name=f"rs_out{grp}")
                     if "cc" in ablate or "rs" in ablate:
                         nc.sync.dma_start(rs_out[:], rs_ins[grp][0:HH, :])
                     else:
                         nc.gpsimd.collective_compute(
                             "ReduceScatter", mybir.AluOpType.add,
                             replica_groups=RG,
                             ins=[rs_ins[grp][:]], outs=[rs_out[:]])
                     rs_outs.append(rs_out)

             stage1(0)
             s2_phase1(0)
             s2_phase1(1)
             stage1(1)
             s2_phase2(0)
             s2_phase2(1)      # issues RS-A
             s2_phase1(2)
             s2_phase1(3)
             s2_phase2(2)
             s2_phase2(3)      # issues RS-B

             # ---- per-half tail: transpose, gates, h', AG ----
             last = (s == STEPS - 1)
             hsh_new, h32_new = [], []
             ag_new = [None, None]
             for k in range(KT):
                 if not last:
                     hr = p_h.tile([128, SH], dt.float16, tag=f"hnr{k}",
                                   name=f"hnr{k}")
                     hsh_new.append(hr)
                 h3 = p_h.tile([128, SH], dt.float32, tag=f"h32{k}",
                               name=f"h32{k}")
                 h32_new.append(h3)

             for X in range(2):
                 cs = slice(X * HH, (X + 1) * HH)
                 an = p_sm.tile([128, D], dt.float16, tag=f"an{X}")
                 nc.sync.dma_start(an[:], rs_outs[X][:])
                 aT = []
                 for kb in range(KT):
                     ptr = p_g.tile([128, 128], dt.float16, tag="gg",
                                    name=f"ptr{kb}")
                     nc.tensor.transpose(
                         ptr[:], an[:, kb * 128:(kb + 1) * 128], identity16[:])
                     a_kb = p_sm.tile([128, HH], dt.float16, tag=f"aT{kb}")
                     nc.vector.tensor_copy(a_kb[:], ptr[:])
                     aT.append(a_kb)

                 def gate_mm(widx, uidx, rhs_u, func, bias_nm, pre=False):
                     Wq, Uq = gw_res[widx], gw_res[uidx]
                     outs = []
                     for f in range(KT):
                         pg = p_g.tile([128, HH], dt.float32, tag="gg",
                                       name=f"g{bias_nm}{f}")
                         if "gru" in ablate:
                             nc.tensor.matmul(pg[:], aT[0][:, 0:128], aT[0][:],
                                              start=True, stop=True)
                         else:
                             if pre:
                                 # preload PSUM with the pre-staged U-term
                                 nc.tensor.matmul(
                                     pg[:], identity16[:],
                                     uP[(bias_nm, X, f)][:],
                                     start=True, stop=False)
                             else:
                                 for k in range(KT):
                                     nc.tensor.matmul(
                                         pg[:], Uq[:, k, f * 128:(f + 1) * 128],
                                         rhs_u[k][:],
                                         start=(k == 0), stop=False)
                             for k in range(KT):
                                 nc.tensor.matmul(
                                     pg[:], Wq[:, k, f * 128:(f + 1) * 128],
                                     aT[k][:],
                                     start=False, stop=(k == KT - 1))
                         og = p_sm.tile([128, HH], dt.float32,
                                        tag=f"g{bias_nm}{f}")
                         nc.scalar.activation(og[:], pg[:], func,
                                              bias=bias_tiles[(bias_nm, f)][:])
                         outs.append(og)
                     return outs

                 z_t = gate_mm(0, 1, None, SIG, "z", pre=True)
                 r_t = gate_mm(2, 3, None, SIG, "r", pre=True)
                 rh = []
                 for k in range(KT):
                     rhk = p_sm.tile([128, HH], dt.float16, tag=f"rh{k}")
                     nc.vector.tensor_mul(rhk[:], r_t[k][:], h32_prev[k][:, cs])
                     rh.append(rhk)
                 ht_t = gate_mm(4, 5, rh, TANH, "h")

                 # h' = h + z * (ht - h) on columns of this half
                 if not last:
                     ag_in = dram.tile([D, HH], dt.float16, tag=f"ag_in{X}",
                                       name=f"ag_in{X}")
                 for k in range(KT):
                     s1 = p_sm.tile([128, HH], dt.float32, tag="gsA")
                     nc.vector.tensor_sub(s1[:], ht_t[k][:], h32_prev[k][:, cs])
                     s2 = p_sm.tile([128, HH], dt.float32, tag="gsB")
                     nc.vector.tensor_mul(s2[:], z_t[k][:], s1[:])
                     nc.vector.tensor_add(h32_new[k][:, cs], h32_prev[k][:, cs],
                                          s2[:])
                     if last:
                         nc.sync.dma_start(out_p[k * 128:(k + 1) * 128, cs],
                                           h32_new[k][:, cs])
                     else:
                         nc.vector.tensor_copy(hsh_new[k][:, cs],
                                               h32_new[k][:, cs])
                         nc.sync.dma_start(ag_in[k * 128:(k + 1) * 128, :],
                                           hsh_new[k][:, cs])

                 if not last:
                     ag_out = dram.tile([NC_CORES * D, HH], dt.float16,
                                        tag=f"ag_out{X}", name=f"ag_out{X}",
                                        addr_space="Shared")
                     if "cc" in ablate or "ag" in ablate:
                         nc.sync.dma_start(ag_out[0:D, :], ag_in[:])
                     else:
                         nc.gpsimd.collective_compute(
                             "AllGather", mybir.AluOpType.bypass,
                             replica_groups=RG,
                             ins=[ag_in[:]], outs=[ag_out[:]])
                     ag_new[X] = ag_out

             if not last:
                 ag_prev = ag_new
                 hsh_prev, h32_prev = hsh_new, h32_new

    nc.finalize()
    return nc


_BUILT = None
TRACE = False
LAST_RESULT = None


_BUILT_R = {}


def _get_built(repeats=1, ablate=()):
    global _BUILT
    key = (repeats, tuple(ablate))
    if key != (1, ()):
        if key not in _BUILT_R:
            _BUILT_R[key] = build(repeats, ablate)
        return _BUILT_R[key]
    if _BUILT is None:
        _BUILT = build()
    return _BUILT


def prepare_in_maps(adjacency, annotations, W_prop, b_prop, Wz, Uz, bz,
                    Wr, Ur, br, Wh, Uh, bh):
    A = np.asarray(adjacency, np.float32)
    ann = np.asarray(annotations, np.float32)
    W_prop = np.asarray(W_prop, np.float32)
    b_prop = np.asarray(b_prop, np.float32)
    gw_all = np.stack([np.asarray(x, np.float32)
                       for x in (Wz, Uz, Wr, Ur, Wh, Uh)]).astype(np.float16)
    bz = np.asarray(bz, np.float32).reshape(D, 1)
    br = np.asarray(br, np.float32).reshape(D, 1)
    bh = np.asarray(bh, np.float32).reshape(D, 1)

    h0 = np.zeros((N, D), np.float32)
    h0[:, :ann.shape[1]] = ann
    h0t = np.ascontiguousarray(h0.T)           # [D, N] fp32
    h0t_r = h0t.astype(np.float16)
    A_T = np.ascontiguousarray(A.T)            # [2E*N, N]

    # shard layout: core c owns node blocks {128c..128c+127, 1024+128c..+127}
    shard_cols = [np.r_[128 * c:128 * c + 128, 1024 + 128 * c:1024 + 128 * c + 128]
                  for c in range(NC_CORES)]
    h0t_ag = np.ascontiguousarray(np.concatenate(
        [h0t_r[:, shard_cols[c]] for c in range(NC_CORES)], axis=0))

    in_maps = []
    for c in range(NC_CORES):
        in_maps.append({
            "at": np.ascontiguousarray(
                A_T[c * N:(c + 1) * N, :]).astype(np.float16),
            "h0t": h0t_ag,
            "h0sr": np.ascontiguousarray(h0t_r[:, shard_cols[c]]),
            "h0s": np.ascontiguousarray(h0t[:, shard_cols[c]]),
            "wc": W_prop[c].astype(np.float16),
            "gw": gw_all,
            "bpc": np.ascontiguousarray(b_prop[c].reshape(1, D)),
            "bzc": bz, "brc": br, "bhc": bh,
        })

    return in_maps


def kernel(**inputs):
    from concourse.bass_utils import run_bass_kernel_spmd

    in_maps = prepare_in_maps(
        **{k: inputs[k] for k in ("adjacency", "annotations", "W_prop", "b_prop",
                                  "Wz", "Uz", "bz", "Wr", "Ur", "br",
                                  "Wh", "Uh", "bh")})
    nc = _get_built()
    res = run_bass_kernel_spmd(nc, in_maps, list(range(NC_CORES)), trace=TRACE)
    global LAST_RESULT
    LAST_RESULT = res
    h = np.empty((N, D), np.float32)
    for c in range(NC_CORES):
        sh = res.results[c]["out"].T           # [SH, D] rows in shard order
        h[128 * c:128 * c + 128] = sh[:128]
        h[1024 + 128 * c:1024 + 128 * c + 128] = sh[128:]
    return h


# revision 39
# speedup vs baseline: 1.0785x; 1.0785x over previous
"""GGNN (gated graph NN) message-passing kernel for 8 Trainium2 NeuronCores.

Sharding: edge-type sharding. Core c owns edge-type block c of the adjacency
matrix (columns c*N..(c+1)*N of the [N, 2E*N] adjacency, pre-transposed on the
host) plus node shard c for the GRU update.

Per step, on core c (node shard split in halves A = block {128c},
B = block {1024+128c} — the blocks the two ReduceScatters deliver):
  stage1: t_c = h @ W_prop[c]                       [N, D]
          (half-A m-tiles emitted first so they only wait on AG-A;
           half-B m-tiles wait on AG-B)
  stage2: partial_a_c = A_cT.T @ t_c                [N, D]  in 4 sub-groups
          of 4 node-tiles (4 PSUM banks), each split into contraction
          phase1 (m=0..7, needs only stage1-A — overlaps in-flight AG-B)
          and phase2 (m=8..15, adds SBUF-staged phase-1 partials on the
          way out). Emission order: s1A, ph1(0,1), s1B, ph2(0), ph2(1)
          [issue RS-A], ph1(2,3), ph2(2), ph2(3) [issue RS-B].
  tail, per half X in {A, B}:
      transpose a_X -> aT_X [D, 128]
      GRU gates on half X (fp16 matmuls, free dim 128)
      h'_X elementwise
      AG-X: AllGather(h'_X^T) -> [8*D, 128]
  Half-A gates run while RS-B is in flight; AG-A runs while half-B gates
  compute; AG-B overlaps next step's stage1-A + stage2-phase1.

Numerics: all matmuls in float16 (full PE rate at any free size; adjacency
0/1 is exact in fp16, weights/states lose ~2^-11 relative); collective
payloads fp16; accumulation fp32 in PSUM; elementwise GRU update in fp32.
"""
import sys
if "/opt/trn_rl_repo" not in sys.path:
    sys.path.insert(0, "/opt/trn_rl_repo")

import numpy as np
import ml_dtypes

NC_CORES = 8
N = 2048          # nodes
D = 512           # state dim
ANN = 256         # annotation dim
STEPS = 5
SH = N // NC_CORES   # 256 nodes per shard
HH = SH // 2         # 128 nodes per half-shard
KT = D // 128        # 4
MT = N // 128        # 16


def build(repeats=1, ablate=()):
    import concourse.bacc as bacc
    import concourse.mybir as mybir
    import concourse.tile as tile
    from concourse.masks import make_identity

    dt = mybir.dt
    nc = bacc.Bacc()
    at_p = nc.declare_dram_parameter("at", [N, N], dt.float16, isOutput=False)
    h0t_p = nc.declare_dram_parameter("h0t", [NC_CORES * D, SH], dt.float16,
                                      isOutput=False)
    h0sr_p = nc.declare_dram_parameter("h0sr", [D, SH], dt.float16, isOutput=False)
    h0s_p = nc.declare_dram_parameter("h0s", [D, SH], dt.float32, isOutput=False)
    wc_p = nc.declare_dram_parameter("wc", [D, D], dt.float16, isOutput=False)
    gw_p = nc.declare_dram_parameter("gw", [6, D, D], dt.float16, isOutput=False)
    bpc_p = nc.declare_dram_parameter("bpc", [1, D], dt.float32, isOutput=False)
    bz_p = nc.declare_dram_parameter("bzc", [D, 1], dt.float32, isOutput=False)
    br_p = nc.declare_dram_parameter("brc", [D, 1], dt.float32, isOutput=False)
    bh_p = nc.declare_dram_parameter("bhc", [D, 1], dt.float32, isOutput=False)
    out_p = nc.declare_dram_parameter("out", [D, SH], dt.float32, isOutput=True)
    RG = [list(range(NC_CORES))]

    from contextlib import ExitStack
    with tile.TileContext(nc) as tc, ExitStack() as stk:
        res = stk.enter_context(tc.tile_pool(name="res", bufs=1))
        # PSUM: bank-granular (8 banks). stage1/stage2 chains use p_mm (5),
        # gate chains + transposes use p_g (3).
        p_mm = stk.enter_context(tc.tile_pool(name="pmm", bufs=5, space="PSUM"))
        p_g = stk.enter_context(tc.tile_pool(name="pg", bufs=3, space="PSUM"))
        p_hc = stk.enter_context(tc.tile_pool(name="phc", bufs=6))
        p_t = stk.enter_context(tc.tile_pool(name="pt", bufs=1))
        p_pp = stk.enter_context(tc.tile_pool(name="ppp", bufs=1))
        p_asb = stk.enter_context(tc.tile_pool(name="pasb", bufs=4))
        p_sm = stk.enter_context(tc.tile_pool(name="psm", bufs=2))
        p_h = stk.enter_context(tc.tile_pool(name="ph", bufs=2))
        dram = stk.enter_context(tc.tile_pool(name="dram", bufs=2, space="DRAM"))

        # ---- setup: constants, weights, adjacency ----
        identity = res.tile([128, 128], dt.float32, tag="identity")
        make_identity(nc, identity[:])
        identity16 = res.tile([128, 128], dt.float16, tag="identity16")
        nc.vector.tensor_copy(identity16[:], identity[:])
        ones = res.tile([1, 128], dt.float32, tag="ones")
        nc.vector.memset(ones[:], 1.0)
        bpc_t = res.tile([1, D], dt.float32, tag="bpc")
        nc.sync.dma_start(bpc_t[:], bpc_p[:])
        pb = p_mm.tile([128, D], dt.float32, tag="mm")
        nc.tensor.matmul(pb[:], ones[:], bpc_t[:], start=True, stop=True)
        bias_bcast = res.tile([128, D], dt.float32, tag="bias_bcast")
        nc.vector.tensor_copy(bias_bcast[:], pb[:])

        bias_tiles = {}
        for nm, par in (("z", bz_p), ("r", br_p), ("h", bh_p)):
            for f in range(KT):
                bt = res.tile([128, 1], dt.float32, tag=f"b{nm}{f}")
                nc.sync.dma_start(bt[:], par[f * 128:(f + 1) * 128, :])
                bias_tiles[(nm, f)] = bt

        wc_t = []
        for k in range(KT):
            w = res.tile([128, D], dt.float16, tag=f"wc{k}")
            nc.sync.dma_start(w[:], wc_p[k * 128:(k + 1) * 128, :])
            wc_t.append(w)

        at_t = []
        for m in range(MT):
            a = res.tile([128, N], dt.float16, tag=f"at{m}")
            nc.sync.dma_start(a[:], at_p[m * 128:(m + 1) * 128, :])
            at_t.append(a)

        # resident GRU weights (fp16), loaded once
        gw_res = []
        for g in range(6):
            w = res.tile([128, KT, D], dt.float16, tag=f"gwr{g}")
            nc.scalar.dma_start(w[:], gw_p[g].rearrange("(k p) f -> p k f", p=128))
            gw_res.append(w)

        import concourse.mybir as _mb
        SIG = _mb.ActivationFunctionType.Sigmoid
        TANH = _mb.ActivationFunctionType.Tanh

        for rep in range(repeats):
          # step-0 h state
          hsh_prev = []   # h^T shard, fp16 (GRU U-term rhs)
          h32_prev = []   # h^T shard, fp32 (elementwise state)
          for k in range(KT):
            hr = p_h.tile([128, SH], dt.float16, tag=f"hnr{k}")
            nc.sync.dma_start(hr[:], h0sr_p[k * 128:(k + 1) * 128, :])
            hsh_prev.append(hr)
            h3 = p_h.tile([128, SH], dt.float32, tag=f"h32{k}")
            nc.sync.dma_start(h3[:], h0s_p[k * 128:(k + 1) * 128, :])
            h32_prev.append(h3)

          ag_prev = None   # pair (agA, agB) of [NC*D, HH] fp16

          for s in range(STEPS):
             # ---- stage 1 + stage 2, software-pipelined ----
             # stage1 half-X m-tiles (mloc=X) only need AG-X of the previous
             # step. stage2 is split into two contraction phases: phase1
             # (m=0..7, needs only stage1-A) runs between stage1-A and
             # stage1-B so it overlaps the in-flight AG-B; phase2 (m=8..15)
             # adds the SBUF-staged phase-1 partials on the way out. This
             # moves RS-A's inputs ~15us earlier.
             t_tiles = [None] * MT

             def stage1(mloc):
                 for mp in range(8):
                     m = mp + 8 * mloc
                     if "s1" in ablate:
                         pt = p_mm.tile([128, D], dt.float32, tag="mm",
                                        name="pt")
                         nc.tensor.matmul(pt[:], wc_t[0][:, 0:128], wc_t[1][:],
                                          start=True, stop=True)
                     else:
                         hc = p_hc.tile([128, KT, 128], dt.float16, tag="hc",
                                        name="hc")
                         if s == 0:
                             blk = h0t_p[512 * mp:512 * (mp + 1),
                                         mloc * HH:(mloc + 1) * HH]
                         else:
                             blk = ag_prev[mloc][512 * mp:512 * (mp + 1), :]
                         nc.sync.dma_start(
                             hc[:], blk.rearrange("(k p) j -> p k j", p=128))
                         pt = p_mm.tile([128, D], dt.float32, tag="mm",
                                        name="pt")
                         for k in range(KT):
                             nc.tensor.matmul(pt[:], hc[:, k, :], wc_t[k][:],
                                              start=(k == 0), stop=(k == KT - 1))
                     tm = p_t.tile([128, D], dt.float16, tag=f"t{m}", name="tm")
                     nc.vector.tensor_add(tm[:], pt[:], bias_bcast[:])
                     t_tiles[m] = tm

             rs_ins = [dram.tile([N // 2, D], dt.float16, tag=f"rs_in{g}",
                                 name=f"rs_in{g}") for g in range(2)]
             rs_outs = []
             pp = {}

             def s2_phase1(sub):
                 if "s2" in ablate:
                     return
                 pas = [p_mm.tile([128, D], dt.float32, tag="mm",
                                  name=f"pa1_{sub}_{i}") for i in range(4)]
                 for m in range(8):
                     for i in range(4):
                         col = sub * 512 + i * 128
                         nc.tensor.matmul(pas[i][:],
                                          at_t[m][:, col:col + 128],
                                          t_tiles[m][:],
                                          start=(m == 0), stop=(m == 7))
                 for i in range(4):
                     ppt = p_pp.tile([128, D], dt.float32, tag=f"pp{sub}{i}",
                                     name=f"pp{sub}{i}")
                     if i % 2 == 0:
                         nc.scalar.copy(ppt[:], pas[i][:])
                     else:
                         nc.vector.tensor_copy(ppt[:], pas[i][:])
                     pp[(sub, i)] = ppt

             def s2_phase2(sub):
                 pas = [p_mm.tile([128, D], dt.float32, tag="mm",
                                  name=f"pa2_{sub}_{i}") for i in range(4)]
                 if "s2" in ablate:
                     for i in range(4):
                         nc.tensor.matmul(pas[i][:], t_tiles[0][:, 0:128],
                                          t_tiles[1][:], start=True, stop=True)
                 else:
                  for m in range(8, MT):
                     for i in range(4):
                         col = sub * 512 + i * 128
                         nc.tensor.matmul(pas[i][:],
                                          at_t[m][:, col:col + 128],
                                          t_tiles[m][:],
                                          start=(m == 8), stop=(m == MT - 1))
                 for i in range(4):
                     asb = p_asb.tile([128, D], dt.float16, tag="asb",
                                      name="asb")
                     if "s2" in ablate:
                         nc.vector.tensor_copy(asb[:], pas[i][:])
                     else:
                         nc.vector.tensor_add(asb[:], pas[i][:],
                                              pp[(sub, i)][:])
                     eng = nc.sync if i % 2 == 0 else nc.scalar
                     row = 512 * (sub % 2) + i * 128
                     eng.dma_start(rs_ins[sub // 2][row:row + 128, :], asb[:])
                 if sub % 2 == 1:
                     grp = sub // 2
                     # RS of this half: core c receives node block grp*1024+128c
                     rs_out = dram.tile([HH, D], dt.float16, tag=f"rs_out{grp}",
                                        name=f"rs_out{grp}")
                     if "cc" in ablate or "rs" in ablate:
                         nc.sync.dma_start(rs_out[:], rs_ins[grp][0:HH, :])
                     else:
                         nc.gpsimd.collective_compute(
                             "ReduceScatter", mybir.AluOpType.add,
                             replica_groups=RG,
                             ins=[rs_ins[grp][:]], outs=[rs_out[:]])
                     rs_outs.append(rs_out)

             stage1(0)
             s2_phase1(0)
             s2_phase1(1)
             stage1(1)
             s2_phase2(0)
             s2_phase2(1)      # issues RS-A
             s2_phase1(2)
             s2_phase1(3)
             s2_phase2(2)
             s2_phase2(3)      # issues RS-B

             # ---- per-half tail: transpose, gates, h', AG ----
             last = (s == STEPS - 1)
             hsh_new, h32_new = [], []
             ag_new = [None, None]
             for k in range(KT):
                 if not last:
                     hr = p_h.tile([128, SH], dt.float16, tag=f"hnr{k}",
                                   name=f"hnr{k}")
                     hsh_new.append(hr)
                 h3 = p_h.tile([128, SH], dt.float32, tag=f"h32{k}",
                               name=f"h32{k}")
                 h32_new.append(h3)

             for X in range(2):
                 cs = slice(X * HH, (X + 1) * HH)
                 an = p_sm.tile([128, D], dt.float16, tag=f"an{X}")
                 nc.sync.dma_start(an[:], rs_outs[X][:])
                 aT = []
                 for kb in range(KT):
                     ptr = p_g.tile([128, 128], dt.float16, tag="gg",
                                    name=f"ptr{kb}")
                     nc.tensor.transpose(
                         ptr[:], an[:, kb * 128:(kb + 1) * 128], identity16[:])
                     a_kb = p_sm.tile([128, HH], dt.float16, tag=f"aT{kb}")
                     nc.vector.tensor_copy(a_kb[:], ptr[:])
                     aT.append(a_kb)

                 def gate_mm(widx, uidx, rhs_u, rhs_u_sl, func, bias_nm):
                     Wq, Uq = gw_res[widx], gw_res[uidx]
                     outs = []
                     for f in range(KT):
                         pg = p_g.tile([128, HH], dt.float32, tag="gg",
                                       name=f"g{bias_nm}{f}")
                         if "gru" in ablate:
                             nc.tensor.matmul(pg[:], aT[0][:, 0:128], aT[0][:],
                                              start=True, stop=True)
                         else:
                             for k in range(KT):
                                 nc.tensor.matmul(
                                     pg[:], Uq[:, k, f * 128:(f + 1) * 128],
                                     rhs_u[k][:, rhs_u_sl] if rhs_u_sl else
                                     rhs_u[k][:],
                                     start=(k == 0), stop=False)
                             for k in range(KT):
                                 nc.tensor.matmul(
                                     pg[:], Wq[:, k, f * 128:(f + 1) * 128],
                                     aT[k][:],
                                     start=False, stop=(k == KT - 1))
                         og = p_sm.tile([128, HH], dt.float32,
                                        tag=f"g{bias_nm}{f}")
                         nc.scalar.activation(og[:], pg[:], func,
                                              bias=bias_tiles[(bias_nm, f)][:])
                         outs.append(og)
                     return outs

                 z_t = gate_mm(0, 1, hsh_prev, cs, SIG, "z")
                 r_t = gate_mm(2, 3, hsh_prev, cs, SIG, "r")
                 rh = []
                 for k in range(KT):
                     rhk = p_sm.tile([128, HH], dt.float16, tag=f"rh{k}")
                     nc.vector.tensor_mul(rhk[:], r_t[k][:], h32_prev[k][:, cs])
                     rh.append(rhk)
                 ht_t = gate_mm(4, 5, rh, None, TANH, "h")

                 # h' = h + z * (ht - h) on columns of this half
                 if not last:
                     ag_in = dram.tile([D, HH], dt.float16, tag=f"ag_in{X}",
                                       name=f"ag_in{X}")
                 for k in range(KT):
                     s1 = p_sm.tile([128, HH], dt.float32, tag="gsA")
                     nc.vector.tensor_sub(s1[:], ht_t[k][:], h32_prev[k][:, cs])
                     s2 = p_sm.tile([128, HH], dt.float32, tag="gsB")
                     nc.vector.tensor_mul(s2[:], z_t[k][:], s1[:])
                     nc.vector.tensor_add(h32_new[k][:, cs], h32_prev[k][:, cs],
                                          s2[:])
                     if last:
                         nc.sync.dma_start(out_p[k * 128:(k + 1) * 128, cs],
                                           h32_new[k][:, cs])
                     else:
                         nc.vector.tensor_copy(hsh_new[k][:, cs],
                                               h32_new[k][:, cs])
                         nc.sync.dma_start(ag_in[k * 128:(k + 1) * 128, :],
                                           hsh_new[k][:, cs])

                 if not last:
                     ag_out = dram.tile([NC_CORES * D, HH], dt.float16,
                                        tag=f"ag_out{X}", name=f"ag_out{X}",
                                        addr_space="Shared")
                     if "cc" in ablate or "ag" in ablate:
                         nc.sync.dma_start(ag_out[0:D, :], ag_in[:])
                     else:
                         nc.gpsimd.collective_compute(
                             "AllGather", mybir.AluOpType.bypass,
                             replica_groups=RG,
                             ins=[ag_in[:]], outs=[ag_out[:]])
                     ag_new[X] = ag_out

             if not last:
                 ag_prev = ag_new
                 hsh_prev, h32_prev = hsh_new, h32_new

    nc.finalize()
    return nc


_BUILT = None
TRACE = False
LAST_RESULT = None


_BUILT_R = {}


def _get_built(repeats=1, ablate=()):
    global _BUILT
    key = (repeats, tuple(ablate))
    if key != (1, ()):
        if key not in _BUILT_R:
            _BUILT_R[key] = build(repeats, ablate)
        return _BUILT_R[key]
    if _BUILT is None:
        _BUILT = build()
    return _BUILT


def prepare_in_maps(adjacency, annotations, W_prop, b_prop, Wz, Uz, bz,
                    Wr, Ur, br, Wh, Uh, bh):
    A = np.asarray(adjacency, np.float32)
    ann = np.asarray(annotations, np.float32)
    W_prop = np.asarray(W_prop, np.float32)
    b_prop = np.asarray(b_prop, np.float32)
    gw_all = np.stack([np.asarray(x, np.float32)
                       for x in (Wz, Uz, Wr, Ur, Wh, Uh)]).astype(np.float16)
    bz = np.asarray(bz, np.float32).reshape(D, 1)
    br = np.asarray(br, np.float32).reshape(D, 1)
    bh = np.asarray(bh, np.float32).reshape(D, 1)

    h0 = np.zeros((N, D), np.float32)
    h0[:, :ann.shape[1]] = ann
    h0t = np.ascontiguousarray(h0.T)           # [D, N] fp32
    h0t_r = h0t.astype(np.float16)
    A_T = np.ascontiguousarray(A.T)            # [2E*N, N]

    # shard layout: core c owns node blocks {128c..128c+127, 1024+128c..+127}
    shard_cols = [np.r_[128 * c:128 * c + 128, 1024 + 128 * c:1024 + 128 * c + 128]
                  for c in range(NC_CORES)]
    h0t_ag = np.ascontiguousarray(np.concatenate(
        [h0t_r[:, shard_cols[c]] for c in range(NC_CORES)], axis=0))

    in_maps = []
    for c in range(NC_CORES):
        in_maps.append({
            "at": np.ascontiguousarray(
                A_T[c * N:(c + 1) * N, :]).astype(np.float16),
            "h0t": h0t_ag,
            "h0sr": np.ascontiguousarray(h0t_r[:, shard_cols[c]]),
            "h0s": np.ascontiguousarray(h0t[:, shard_cols[c]]),
            "wc": W_prop[c].astype(np.float16),
            "gw": gw_all,
            "bpc": np.ascontiguousarray(b_prop[c].reshape(1, D)),
            "bzc": bz, "brc": br, "bhc": bh,
        })

    return in_maps


def kernel(**inputs):
    from concourse.bass_utils import run_bass_kernel_spmd

    in_maps = prepare_in_maps(
        **{k: inputs[k] for k in ("adjacency", "annotations", "W_prop", "b_prop",
                                  "Wz", "Uz", "bz", "Wr", "Ur", "br",
                                  "Wh", "Uh", "bh")})
    nc = _get_built()
    res = run_bass_kernel_spmd(nc, in_maps, list(range(NC_CORES)), trace=TRACE)
    global LAST_RESULT
    LAST_RESULT = res
    h = np.empty((N, D), np.float32)
    for c in range(NC_CORES):
        sh = res.results[c]["out"].T           # [SH, D] rows in shard order
        h[128 * c:128 * c + 128] = sh[:128]
        h[1024 + 128 * c:1024 + 128 * c + 128] = sh[128:]
    return h


# revision 41
# speedup vs baseline: 1.1315x; 1.0491x over previous
"""GGNN (gated graph NN) message-passing kernel for 8 Trainium2 NeuronCores.

Sharding: edge-type sharding. Core c owns edge-type block c of the adjacency
matrix (columns c*N..(c+1)*N of the [N, 2E*N] adjacency, pre-transposed on the
host) plus node shard c for the GRU update.

Per step, on core c (node shard split in halves A = block {128c},
B = block {1024+128c} — the blocks the two ReduceScatters deliver):
  stage1: t_c = h @ W_prop[c]                       [N, D]
          (half-A m-tiles emitted first so they only wait on AG-A;
           half-B m-tiles wait on AG-B)
  stage2: partial_a_c = A_cT.T @ t_c                [N, D]  in 4 sub-groups
          of 4 node-tiles (4 PSUM banks), each split into contraction
          phase1 (m=0..7, needs only stage1-A — overlaps in-flight AG-B)
          and phase2 (m=8..15, adds SBUF-staged phase-1 partials on the
          way out). Emission order: s1A, ph1(0,1), s1B, ph2(0), ph2(1)
          [issue RS-A], ph1(2,3), ph2(2), ph2(3) [issue RS-B].
  tail, per half X in {A, B}:
      transpose a_X -> aT_X [D, 128]
      GRU gates on half X (fp16 matmuls, free dim 128)
      h'_X elementwise
      AG-X: AllGather(h'_X^T) -> [8*D, 128]
  Half-A gates run while RS-B is in flight; AG-A runs while half-B gates
  compute; AG-B overlaps next step's stage1-A + stage2-phase1.

Numerics: all matmuls in float16 (full PE rate at any free size; adjacency
0/1 is exact in fp16, weights/states lose ~2^-11 relative); collective
payloads fp16; accumulation fp32 in PSUM; elementwise GRU update in fp32.
"""
import sys
if "/opt/trn_rl_repo" not in sys.path:
    sys.path.insert(0, "/opt/trn_rl_repo")

import numpy as np
import ml_dtypes

NC_CORES = 8
N = 2048          # nodes
D = 512           # state dim
ANN = 256         # annotation dim
STEPS = 5
SH = N // NC_CORES   # 256 nodes per shard
HH = SH // 2         # 128 nodes per half-shard
KT = D // 128        # 4
MT = N // 128        # 16


def build(repeats=1, ablate=()):
    import concourse.bacc as bacc
    import concourse.mybir as mybir
    import concourse.tile as tile
    from concourse.masks import make_identity

    dt = mybir.dt
    nc = bacc.Bacc()
    at_p = nc.declare_dram_parameter("at", [N, N], dt.float16, isOutput=False)
    h0t_p = nc.declare_dram_parameter("h0t", [NC_CORES * D, SH], dt.float16,
                                      isOutput=False)
    h0sr_p = nc.declare_dram_parameter("h0sr", [D, SH], dt.float16, isOutput=False)
    h0s_p = nc.declare_dram_parameter("h0s", [D, SH], dt.float32, isOutput=False)
    wc_p = nc.declare_dram_parameter("wc", [D, D], dt.float16, isOutput=False)
    gw_p = nc.declare_dram_parameter("gw", [6, D, D], dt.float16, isOutput=False)
    bpc_p = nc.declare_dram_parameter("bpc", [1, D], dt.float32, isOutput=False)
    bz_p = nc.declare_dram_parameter("bzc", [D, 1], dt.float32, isOutput=False)
    br_p = nc.declare_dram_parameter("brc", [D, 1], dt.float32, isOutput=False)
    bh_p = nc.declare_dram_parameter("bhc", [D, 1], dt.float32, isOutput=False)
    out_p = nc.declare_dram_parameter("out", [D, SH], dt.float32, isOutput=True)
    RG = [list(range(NC_CORES))]

    from contextlib import ExitStack
    with tile.TileContext(nc) as tc, ExitStack() as stk:
        res = stk.enter_context(tc.tile_pool(name="res", bufs=1))
        # PSUM: bank-granular (8 banks). stage1/stage2 chains use p_mm (5),
        # gate chains + transposes use p_g (3).
        p_mm = stk.enter_context(tc.tile_pool(name="pmm", bufs=5, space="PSUM"))
        p_g = stk.enter_context(tc.tile_pool(name="pg", bufs=3, space="PSUM"))
        p_hc = stk.enter_context(tc.tile_pool(name="phc", bufs=6))
        p_t = stk.enter_context(tc.tile_pool(name="pt", bufs=1))
        p_pp = stk.enter_context(tc.tile_pool(name="ppp", bufs=1))
        p_asb = stk.enter_context(tc.tile_pool(name="pasb", bufs=4))
        p_sm = stk.enter_context(tc.tile_pool(name="psm", bufs=2))
        p_h = stk.enter_context(tc.tile_pool(name="ph", bufs=2))
        dram = stk.enter_context(tc.tile_pool(name="dram", bufs=2, space="DRAM"))

        # ---- setup: constants, weights, adjacency ----
        identity = res.tile([128, 128], dt.float32, tag="identity")
        make_identity(nc, identity[:])
        identity16 = res.tile([128, 128], dt.float16, tag="identity16")
        nc.vector.tensor_copy(identity16[:], identity[:])
        ones = res.tile([1, 128], dt.float32, tag="ones")
        nc.vector.memset(ones[:], 1.0)
        bpc_t = res.tile([1, D], dt.float32, tag="bpc")
        nc.sync.dma_start(bpc_t[:], bpc_p[:])
        pb = p_mm.tile([128, D], dt.float32, tag="mm")
        nc.tensor.matmul(pb[:], ones[:], bpc_t[:], start=True, stop=True)
        bias_bcast = res.tile([128, D], dt.float32, tag="bias_bcast")
        nc.vector.tensor_copy(bias_bcast[:], pb[:])

        bias_tiles = {}
        for nm, par in (("z", bz_p), ("r", br_p), ("h", bh_p)):
            for f in range(KT):
                bt = res.tile([128, 1], dt.float32, tag=f"b{nm}{f}")
                nc.sync.dma_start(bt[:], par[f * 128:(f + 1) * 128, :])
                bias_tiles[(nm, f)] = bt

        wc_t = []
        for k in range(KT):
            w = res.tile([128, D], dt.float16, tag=f"wc{k}")
            nc.sync.dma_start(w[:], wc_p[k * 128:(k + 1) * 128, :])
            wc_t.append(w)

        at_t = []
        for m in range(MT):
            a = res.tile([128, N], dt.float16, tag=f"at{m}")
            nc.sync.dma_start(a[:], at_p[m * 128:(m + 1) * 128, :])
            at_t.append(a)

        # resident GRU weights (fp16), loaded once
        gw_res = []
        for g in range(6):
            w = res.tile([128, KT, D], dt.float16, tag=f"gwr{g}")
            nc.scalar.dma_start(w[:], gw_p[g].rearrange("(k p) f -> p k f", p=128))
            gw_res.append(w)

        import concourse.mybir as _mb
        SIG = _mb.ActivationFunctionType.Sigmoid
        TANH = _mb.ActivationFunctionType.Tanh

        for rep in range(repeats):
          # step-0 h state
          hsh_prev = []   # h^T shard, fp16 (GRU U-term rhs)
          h32_prev = []   # h^T shard, fp32 (elementwise state)
          for k in range(KT):
            hr = p_h.tile([128, SH], dt.float16, tag=f"hnr{k}")
            nc.sync.dma_start(hr[:], h0sr_p[k * 128:(k + 1) * 128, :])
            hsh_prev.append(hr)
            h3 = p_h.tile([128, SH], dt.float32, tag=f"h32{k}")
            nc.sync.dma_start(h3[:], h0s_p[k * 128:(k + 1) * 128, :])
            h32_prev.append(h3)

          ag_prev = None   # pair (agA, agB) of [NC*D, HH] fp16

          for s in range(STEPS):
             # ---- stage 1 + stage 2, software-pipelined ----
             # stage1 half-X m-tiles (mloc=X) only need AG-X of the previous
             # step. stage2 is split into two contraction phases: phase1
             # (m=0..7, needs only stage1-A) runs between stage1-A and
             # stage1-B so it overlaps the in-flight AG-B; phase2 (m=8..15)
             # adds the SBUF-staged phase-1 partials on the way out. This
             # moves RS-A's inputs ~15us earlier.
             t_tiles = [None] * MT

             def stage1(mloc):
                 for mp in range(8):
                     m = mp + 8 * mloc
                     if "s1" in ablate:
                         pt = p_mm.tile([128, D], dt.float32, tag="mm",
                                        name="pt")
                         nc.tensor.matmul(pt[:], wc_t[0][:, 0:128], wc_t[1][:],
                                          start=True, stop=True)
                     else:
                         hc = p_hc.tile([128, KT, 128], dt.float16, tag="hc",
                                        name="hc")
                         if s == 0:
                             blk = h0t_p[512 * mp:512 * (mp + 1),
                                         mloc * HH:(mloc + 1) * HH]
                         else:
                             blk = ag_prev[mloc][512 * mp:512 * (mp + 1), :]
                         nc.sync.dma_start(
                             hc[:], blk.rearrange("(k p) j -> p k j", p=128))
                         pt = p_mm.tile([128, D], dt.float32, tag="mm",
                                        name="pt")
                         for k in range(KT):
                             nc.tensor.matmul(pt[:], hc[:, k, :], wc_t[k][:],
                                              start=(k == 0), stop=(k == KT - 1))
                     tm = p_t.tile([128, D], dt.float16, tag=f"t{m}", name="tm")
                     nc.vector.tensor_add(tm[:], pt[:], bias_bcast[:])
                     t_tiles[m] = tm

             rs_ins = [dram.tile([N // 2, D], dt.float16, tag=f"rs_in{g}",
                                 name=f"rs_in{g}") for g in range(2)]
             rs_outs = []
             pp = {}

             def s2_phase1(sub):
                 if "s2" in ablate:
                     return
                 # i-major: each chain finishes ~10us before the last, so its
                 # PSUM drain overlaps the remaining chains' matmuls
                 for i in range(4):
                     pa = p_mm.tile([128, D], dt.float32, tag="mm",
                                    name=f"pa1_{sub}_{i}")
                     col = sub * 512 + i * 128
                     for m in range(8):
                         nc.tensor.matmul(pa[:],
                                          at_t[m][:, col:col + 128],
                                          t_tiles[m][:],
                                          start=(m == 0), stop=(m == 7))
                     ppt = p_pp.tile([128, D], dt.float32, tag=f"pp{sub}{i}",
                                     name=f"pp{sub}{i}")
                     if i % 2 == 0:
                         nc.scalar.copy(ppt[:], pa[:])
                     else:
                         nc.vector.tensor_copy(ppt[:], pa[:])
                     pp[(sub, i)] = ppt

             def s2_phase2(sub):
                 for i in range(4):
                     pa = p_mm.tile([128, D], dt.float32, tag="mm",
                                    name=f"pa2_{sub}_{i}")
                     if "s2" in ablate:
                         nc.tensor.matmul(pa[:], t_tiles[0][:, 0:128],
                                          t_tiles[1][:], start=True, stop=True)
                     else:
                         col = sub * 512 + i * 128
                         for m in range(8, MT):
                             nc.tensor.matmul(pa[:],
                                              at_t[m][:, col:col + 128],
                                              t_tiles[m][:],
                                              start=(m == 8), stop=(m == MT - 1))
                     asb = p_asb.tile([128, D], dt.float16, tag="asb",
                                      name="asb")
                     if "s2" in ablate:
                         nc.vector.tensor_copy(asb[:], pa[:])
                     else:
                         nc.vector.tensor_add(asb[:], pa[:], pp[(sub, i)][:])
                     eng = nc.sync if i % 2 == 0 else nc.scalar
                     row = 512 * (sub % 2) + i * 128
                     eng.dma_start(rs_ins[sub // 2][row:row + 128, :], asb[:])
                 if sub % 2 == 1:
                     grp = sub // 2
                     # RS of this half: core c receives node block grp*1024+128c
                     rs_out = dram.tile([HH, D], dt.float16, tag=f"rs_out{grp}",
                                        name=f"rs_out{grp}")
                     if "cc" in ablate or "rs" in ablate:
                         nc.sync.dma_start(rs_out[:], rs_ins[grp][0:HH, :])
                     else:
                         nc.gpsimd.collective_compute(
                             "ReduceScatter", mybir.AluOpType.add,
                             replica_groups=RG,
                             ins=[rs_ins[grp][:]], outs=[rs_out[:]])
                     rs_outs.append(rs_out)

             stage1(0)
             s2_phase1(0)
             s2_phase1(1)
             stage1(1)
             s2_phase2(0)
             s2_phase2(1)      # issues RS-A
             s2_phase1(2)
             s2_phase1(3)
             s2_phase2(2)
             s2_phase2(3)      # issues RS-B

             # ---- per-half tail: transpose, gates, h', AG ----
             last = (s == STEPS - 1)
             hsh_new, h32_new = [], []
             ag_new = [None, None]
             for k in range(KT):
                 if not last:
                     hr = p_h.tile([128, SH], dt.float16, tag=f"hnr{k}",
                                   name=f"hnr{k}")
                     hsh_new.append(hr)
                 h3 = p_h.tile([128, SH], dt.float32, tag=f"h32{k}",
                               name=f"h32{k}")
                 h32_new.append(h3)

             for X in range(2):
                 cs = slice(X * HH, (X + 1) * HH)
                 an = p_sm.tile([128, D], dt.float16, tag=f"an{X}")
                 nc.sync.dma_start(an[:], rs_outs[X][:])
                 aT = []
                 for kb in range(KT):
                     ptr = p_g.tile([128, 128], dt.float16, tag="gg",
                                    name=f"ptr{kb}")
                     nc.tensor.transpose(
                         ptr[:], an[:, kb * 128:(kb + 1) * 128], identity16[:])
                     a_kb = p_sm.tile([128, HH], dt.float16, tag=f"aT{kb}")
                     nc.vector.tensor_copy(a_kb[:], ptr[:])
                     aT.append(a_kb)

                 def gate_mm(widx, uidx, rhs_u, rhs_u_sl, func, bias_nm):
                     Wq, Uq = gw_res[widx], gw_res[uidx]
                     outs = []
                     for f in range(KT):
                         pg = p_g.tile([128, HH], dt.float32, tag="gg",
                                       name=f"g{bias_nm}{f}")
                         if "gru" in ablate:
                             nc.tensor.matmul(pg[:], aT[0][:, 0:128], aT[0][:],
                                              start=True, stop=True)
                         else:
                             for k in range(KT):
                                 nc.tensor.matmul(
                                     pg[:], Uq[:, k, f * 128:(f + 1) * 128],
                                     rhs_u[k][:, rhs_u_sl] if rhs_u_sl else
                                     rhs_u[k][:],
                                     start=(k == 0), stop=False)
                             for k in range(KT):
                                 nc.tensor.matmul(
                                     pg[:], Wq[:, k, f * 128:(f + 1) * 128],
                                     aT[k][:],
                                     start=False, stop=(k == KT - 1))
                         og = p_sm.tile([128, HH], dt.float32,
                                        tag=f"g{bias_nm}{f}")
                         nc.scalar.activation(og[:], pg[:], func,
                                              bias=bias_tiles[(bias_nm, f)][:])
                         outs.append(og)
                     return outs

                 z_t = gate_mm(0, 1, hsh_prev, cs, SIG, "z")
                 r_t = gate_mm(2, 3, hsh_prev, cs, SIG, "r")
                 rh = []
                 for k in range(KT):
                     rhk = p_sm.tile([128, HH], dt.float16, tag=f"rh{k}")
                     nc.vector.tensor_mul(rhk[:], r_t[k][:], h32_prev[k][:, cs])
                     rh.append(rhk)
                 ht_t = gate_mm(4, 5, rh, None, TANH, "h")

                 # h' = h + z * (ht - h) on columns of this half
                 if not last:
                     ag_in = dram.tile([D, HH], dt.float16, tag=f"ag_in{X}",
                                       name=f"ag_in{X}")
                 for k in range(KT):
                     s1 = p_sm.tile([128, HH], dt.float32, tag="gsA")
                     nc.vector.tensor_sub(s1[:], ht_t[k][:], h32_prev[k][:, cs])
                     s2 = p_sm.tile([128, HH], dt.float32, tag="gsB")
                     nc.vector.tensor_mul(s2[:], z_t[k][:], s1[:])
                     nc.vector.tensor_add(h32_new[k][:, cs], h32_prev[k][:, cs],
                                          s2[:])
                     if last:
                         nc.sync.dma_start(out_p[k * 128:(k + 1) * 128, cs],
                                           h32_new[k][:, cs])
                     else:
                         nc.vector.tensor_copy(hsh_new[k][:, cs],
                                               h32_new[k][:, cs])
                         nc.sync.dma_start(ag_in[k * 128:(k + 1) * 128, :],
                                           hsh_new[k][:, cs])

                 if not last:
                     ag_out = dram.tile([NC_CORES * D, HH], dt.float16,
                                        tag=f"ag_out{X}", name=f"ag_out{X}",
                                        addr_space="Shared")
                     if "cc" in ablate or "ag" in ablate:
                         nc.sync.dma_start(ag_out[0:D, :], ag_in[:])
                     else:
                         nc.gpsimd.collective_compute(
                             "AllGather", mybir.AluOpType.bypass,
                             replica_groups=RG,
                             ins=[ag_in[:]], outs=[ag_out[:]])
                     ag_new[X] = ag_out

             if not last:
                 ag_prev = ag_new
                 hsh_prev, h32_prev = hsh_new, h32_new

    nc.finalize()
    return nc


_BUILT = None
TRACE = False
LAST_RESULT = None


_BUILT_R = {}


def _get_built(repeats=1, ablate=()):
    global _BUILT
    key = (repeats, tuple(ablate))
    if key != (1, ()):
        if key not in _BUILT_R:
            _BUILT_R[key] = build(repeats, ablate)
        return _BUILT_R[key]
    if _BUILT is None:
        _BUILT = build()
    return _BUILT


def prepare_in_maps(adjacency, annotations, W_prop, b_prop, Wz, Uz, bz,
                    Wr, Ur, br, Wh, Uh, bh):
    A = np.asarray(adjacency, np.float32)
    ann = np.asarray(annotations, np.float32)
    W_prop = np.asarray(W_prop, np.float32)
    b_prop = np.asarray(b_prop, np.float32)
    gw_all = np.stack([np.asarray(x, np.float32)
                       for x in (Wz, Uz, Wr, Ur, Wh, Uh)]).astype(np.float16)
    bz = np.asarray(bz, np.float32).reshape(D, 1)
    br = np.asarray(br, np.float32).reshape(D, 1)
    bh = np.asarray(bh, np.float32).reshape(D, 1)

    h0 = np.zeros((N, D), np.float32)
    h0[:, :ann.shape[1]] = ann
    h0t = np.ascontiguousarray(h0.T)           # [D, N] fp32
    h0t_r = h0t.astype(np.float16)
    A_T = np.ascontiguousarray(A.T)            # [2E*N, N]

    # shard layout: core c owns node blocks {128c..128c+127, 1024+128c..+127}
    shard_cols = [np.r_[128 * c:128 * c + 128, 1024 + 128 * c:1024 + 128 * c + 128]
                  for c in range(NC_CORES)]
    h0t_ag = np.ascontiguousarray(np.concatenate(
        [h0t_r[:, shard_cols[c]] for c in range(NC_CORES)], axis=0))

    in_maps = []
    for c in range(NC_CORES):
        in_maps.append({
            "at": np.ascontiguousarray(
                A_T[c * N:(c + 1) * N, :]).astype(np.float16),
            "h0t": h0t_ag,
            "h0sr": np.ascontiguousarray(h0t_r[:, shard_cols[c]]),
            "h0s": np.ascontiguousarray(h0t[:, shard_cols[c]]),
            "wc": W_prop[c].astype(np.float16),
            "gw": gw_all,
            "bpc": np.ascontiguousarray(b_prop[c].reshape(1, D)),
            "bzc": bz, "brc": br, "bhc": bh,
        })

    return in_maps


def kernel(**inputs):
    from concourse.bass_utils import run_bass_kernel_spmd

    in_maps = prepare_in_maps(
        **{k: inputs[k] for k in ("adjacency", "annotations", "W_prop", "b_prop",
                                  "Wz", "Uz", "bz", "Wr", "Ur", "br",
                                  "Wh", "Uh", "bh")})
    nc = _get_built()
    res = run_bass_kernel_spmd(nc, in_maps, list(range(NC_CORES)), trace=TRACE)
    global LAST_RESULT
    LAST_RESULT = res
    h = np.empty((N, D), np.float32)
    for c in range(NC_CORES):
        sh = res.results[c]["out"].T           # [SH, D] rows in shard order
        h[128 * c:128 * c + 128] = sh[:128]
        h[1024 + 128 * c:1024 + 128 * c + 128] = sh[128:]
    return h
